# revision 1
# baseline (speedup 1.0000x reference)
"""Bass/Trainium2 kernel for nn_EnhancedBianGuaAttention_76055280878201.

Contract: kernel(**inputs) takes the FULL unsharded inputs (as produced by
reference.setup_inputs()) and returns the FULL (B, T, D) output.

Sharding: 8 cores = 2 batches x 4 head-groups (4 heads each).  Each core:
  - computes q/k/v projections (transposed layout) for its 4 heads from x[b]^T
  - computes u^T = tanh([hex_w; q6_w] @ x^T)  (12, T) and per-head
    A_h^T = B_h^T u^T where B_h = diag(lam/2 * I6, sig(scale)*2/6 * Mh)
    so that the full score bias is bias[i,j] = sum_c u[j,c] A_h[i,c]
  - flash-style causal attention, scores computed transposed (keys on the
    partition axis):  E^T[j,i] = exp(alpha*tanh(qk/beta) + bias^T),
    masked with affine_select; denominators via a ones-column appended to V
  - its 4 heads' slice of the output projection, written transposed
Host side: sums the 4 partial outputs per batch and transposes.

Precision: the projection/score/AV chains run in SC_DT (bf16 by default; PE
full rate, cheap ACT/DVE, half DMA).  The output projection and the softmax
normalization stay in fp32r/fp32 (PSUM accumulation is always fp32).
"""

import os
import sys

import numpy as np

for _p in ("/opt/trn_rl_repo", "/root/.axon_site/_ro/trn_rl_repo"):
    if os.path.isdir(_p) and _p not in sys.path:
        sys.path.append(_p)

import ml_dtypes
import concourse.bacc as bacc
import concourse.mybir as mybir
import concourse.tile as tile
from concourse.bass_utils import run_bass_kernel_spmd

B, T, D, H, NT = 2, 2048, 1024, 16, 7
HD = D // H          # 64
TEMP = 0.5
NCORES = 8
HPC = 4              # heads per core
CPB = NCORES // B    # cores per batch (4)
TC = 512             # query-chunk size
NTC = T // TC        # 4
JB = 128             # key-block size
NJB = T // JB        # 16
KC = D // 128        # contraction chunks for the projections (8)

F32 = mybir.dt.float32
F32R = mybir.dt.float32r
BF16 = mybir.dt.bfloat16
Act = mybir.ActivationFunctionType
Alu = mybir.AluOpType

# score-chain dtype: BF16 (fast) or F32R (precise)
USE_BF16 = os.environ.get("KERNEL_F32R") != "1"


def _emit(nc, tc_, dr, alpha, inv_beta):
    """Emit the per-core program. dr: dict of DRAM APs."""
    SC = BF16 if USE_BF16 else F32R
    xT_r = dr["xT"].rearrange("(c p) t -> c p t", p=128)       # (8,128,T)
    xT_m = dr["xT"].rearrange("(c p) t -> p c t", p=128)       # (128,8,T)
    wqkv_r = dr["wqkvT"].rearrange("(c p) m -> c p m", p=128)  # (8,128,768)
    whq_r = dr["whqT"].rearrange("(c p) w -> p c w", p=128)    # (128,8,12)

    with (
        tc_.tile_pool(name="persist", bufs=1) as pp,
        tc_.tile_pool(name="work", bufs=1) as wp,
        tc_.tile_pool(name="psum", bufs=1, space="PSUM") as sp,
    ):
        # ---- constants / weights (pre-rounded on host) --------------
        ident = pp.tile([128, 128], SC)
        nc.sync.dma_start(out=ident[:], in_=dr["ident"])
        bTz = pp.tile([128, 128], SC)
        nc.sync.dma_start(out=bTz[:], in_=dr["bT"])
        owt = [pp.tile([128, D], F32R, name=f"owt{i}") for i in range(2)]
        whq = wp.tile([128, KC, 128], SC)
        nc.sync.dma_start(out=whq[:], in_=whq_r)
        # chunk-0 x tiles load before the bulk projection weights so the
        # first matmul chain (pu: whq @ xt) can start as early as possible
        xt0 = [
            wp.tile([128, TC], SC, name=f"xt{c}", tag=f"xt{c}", bufs=3)
            for c in range(KC)
        ]
        for c in range(KC):
            nc.sync.dma_start(out=xt0[c][:], in_=xT_r[c, :, 0:TC])
        wq = [wp.tile([128, 3 * HPC * HD], SC, name=f"wq{c}") for c in range(KC)]
        for c in range(KC):
            nc.sync.dma_start(out=wq[c][:], in_=wqkv_r[c])
        ones_col = pp.tile([128, 1], F32)
        nc.gpsimd.memset(ones_col[:], 1.0)
        # trinegT[k, j] = -30 where query k < key j (causal mask, applied
        # additively into the bias psum via a PE matmul against ident)
        trinegT = pp.tile([128, 128], SC)
        nc.sync.dma_start(out=trinegT[:], in_=dr["trinegT"])

        # ---- persistent activations ---------------------------------
        # All score-chain matmuls are zero-padded to a full K=128
        # contraction: the PE clock-gate (HAM) watches array activity, and
        # low-K matmuls read as idle and get the clock halved.
        _ms = nc.gpsimd if USE_BF16 else nc.vector
        uz = pp.tile([128, T], SC)
        _ms.memset(uz[:], 0.0)
        aT = [pp.tile([128, T], SC, name=f"aT{h}") for h in range(HPC)]
        for h in range(HPC):
            _ms.memset(aT[h][:], 0.0)
        # q zero-padded per head (other head's 64 rows are 0);
        # k keeps 2 heads per tile: rows h%2*64 .. +64
        qz = [pp.tile([128, T], SC, name=f"qz{h}") for h in range(HPC)]
        for h in range(HPC):
            _ms.memset(qz[h][:], 0.0)
        kt = [pp.tile([128, T], SC, name=f"kt{i}") for i in range(2)]
        vt = [pp.tile([128, T], SC, name=f"vt{i}") for i in range(2)]
        qkv_tiles = [None, None] + kt + vt  # oc: q01,q23,k01,k23,v01,v23
        # V' per head: natural layout + ones column, 16 blocks of (128, 65).
        # The ones columns are constant: set once via a strided memset
        # instead of 64 per-block copies in the hot loop.
        vp = [pp.tile([128, NJB, HD + 1], SC, name=f"vp{h}")
              for h in range(HPC)]
        for h in range(HPC):
            nc.vector.memset(vp[h][:, :, HD:HD + 1], 1.0)
        # normalized attention out (transposed, f32r), 2 heads per tile
        ao = [pp.tile([128, T], F32R, name=f"ao{i}") for i in range(2)]
        # K=128-padded denominator-broadcast operands (kills the K=1
        # matmuls that re-throttle the PE clock gate around each norm)
        e0z2 = pp.tile([128, 128], F32R)
        dnz = [pp.tile([128, TC], F32R, name=f"dnz{i}") for i in range(2)]

        # ---- per-t-chunk projection + V'-build emitters -------------
        def proj_chunk(t4, xt=None):
            sl = slice(t4 * TC, (t4 + 1) * TC)
            if xt is None:
                # single merged tile + one dma_start: each dma_start costs
                # ~310ns serialized on the sync engine, so batch the 8
                # contraction chunks into one descriptor
                xtm = wp.tile([128, KC, TC], SC, name="xtm", tag="xtm",
                              bufs=2)
                nc.sync.dma_start(out=xtm[:], in_=xT_m[:, :, sl])
                xt = [xtm[:, c, :] for c in range(KC)]

            # u^T chunk: u replicated into 4 row bands (12h..12h+12), so
            # one block-diagonal matmul makes all 4 heads' A_h^T at once
            # and each head's bias operands live in their own band
            pu = sp.tile([128, TC], F32, name="pu", tag="mm", bufs=4)
            for c in range(KC):
                nc.tensor.matmul(pu[:], whq[:, c, :], xt[c][:],
                                 start=(c == 0), stop=(c == KC - 1))
            nc.scalar.activation(uz[:, sl], pu[:], Act.Tanh)

            # A_h^T for all 4 heads in one matmul (block-diagonal bTz);
            # head h's band starts at partition 32*h (engine APs need
            # 32-aligned bases)
            pa = sp.tile([128, TC], F32, name="pa", tag="mm", bufs=4)
            nc.tensor.matmul(pa[:], bTz[:], uz[:, sl],
                             start=True, stop=True)
            for h in range(HPC):
                nc.vector.tensor_copy(aT[h][32 * h:32 * h + 12, sl],
                                      pa[32 * h:32 * h + 12, :])

            # qkv projections; q chunks are split into per-head
            # zero-padded tiles (see K=128 note above)
            for oc in range(6):
                pq = sp.tile([128, TC], F32, name="pq", tag="mm", bufs=4)
                for c in range(KC):
                    nc.tensor.matmul(pq[:], wq[c][:, oc * 128:(oc + 1) * 128],
                                     xt[c][:], start=(c == 0), stop=(c == KC - 1))
                if oc < 2:
                    nc.vector.tensor_copy(qz[2 * oc][0:HD, sl], pq[0:HD, :])
                    nc.vector.tensor_copy(qz[2 * oc + 1][HD:128, sl],
                                          pq[HD:128, :])
                else:
                    nc.vector.tensor_copy(qkv_tiles[oc][:, sl], pq[:])

        def vp_build(t4):
            # one full-width transpose per head PAIR and key block: output
            # columns 0-63 are the even head's features, 64-127 the odd's
            for hp in (0, 2):
                for tb in range(4 * t4, 4 * t4 + 4):
                    pv = sp.tile([128, 128], SC, name="pv", tag="mm", bufs=4)
                    nc.tensor.transpose(
                        pv[:], vt[hp // 2][:, tb * JB:(tb + 1) * JB],
                        ident[:])
                    nc.vector.tensor_copy(
                        vp[hp][:, tb, 0:HD], pv[:, 0:HD])
                    nc.vector.tensor_copy(
                        vp[hp + 1][:, tb, 0:HD], pv[:, HD:128])

        # ---- attention (ic outer, head-pairs interleaved) -----------
        # Two independent (h, ic) dependency chains in flight keep every
        # engine fed; the output projection for t-chunk ic is emitted as
        # soon as all 4 heads finish that ic, overlapping phase D with C.
        def attn_tile(h, ic, jb, po, pair):
            # Full (non-diagonal) tiles arrive in pairs sharing a (128, 2*TC)
            # t1/ee tile so the exp runs once per pair (halves ACT op
            # overhead).  pair = (t1p, eep, member) or None for diagonal
            # tiles, which process only their live columns [off, TC).
            njb = 4 * (ic + 1)
            jsl = slice(jb * JB, (jb + 1) * JB)
            off = max(0, jb * JB - ic * TC)
            w = TC - off
            csl = slice(ic * TC + off, (ic + 1) * TC)
            pr = sp.tile([128, TC], F32, name="pr", tag="mm", bufs=4)
            nc.tensor.matmul(pr[:, :w], kt[h // 2][:, jsl], qz[h][:, csl],
                             start=True, stop=True)
            if pair is None:
                t1 = wp.tile([128, TC], SC, name="t1", tag="t1", bufs=12)
                ee = wp.tile([128, TC], SC, name="ee", tag="ee", bufs=12)
                t1v, eev = t1[:, :w], ee[:, :w]
            else:
                t1p, eep, q = pair
                qsl = slice(q * TC, (q + 1) * TC)
                t1v, eev = t1p[:, qsl], eep[:, qsl]
            # bias matmul in its own psum slot, off the tanh critical path
            # (PE fills the tanh gap with it; the slot frees right after
            # the stt instead of holding qk->stt)
            diag = jb >= 4 * ic
            prb2 = sp.tile([128, TC], F32, name="prb2", tag="mm", bufs=4)
            nc.tensor.matmul(prb2[:, :w], uz[:, jsl], aT[h][:, csl],
                             start=True, stop=not diag)
            if diag:
                # additive causal mask: pr[j, k] += -30 where k < j, so the
                # masked scores exp to ~0 (no post-exp DVE multiply needed)
                nc.tensor.matmul(prb2[:, 0:JB], trinegT[:], ident[:, 0:JB],
                                 start=False, stop=True)
            nc.scalar.activation(t1v, pr[:, :w], Act.Tanh, scale=inv_beta)
            nc.vector.scalar_tensor_tensor(
                t1v, t1v, alpha, prb2[:, :w], op0=Alu.mult, op1=Alu.add)
            if pair is None:
                nc.scalar.activation(eev, t1v, Act.Exp)
                attn_av(h, ic, jb, po, eev)
            elif pair[2] == 1:
                nc.scalar.activation(eep[:], t1p[:], Act.Exp)

        def attn_av(h, ic, jb, po, eev):
            njb = 4 * (ic + 1)
            off = max(0, jb * JB - ic * TC)
            nc.tensor.matmul(
                po[:, off:], vp[h][:, jb, :],
                eev, start=(jb == 0), stop=(jb == njb - 1))

        def attn_norm(hp, ic, po_a, po_b):
            # normalize both heads of the pair at once: denominators of
            # po_a/po_b are packed into rows 0 and 64 of dz, broadcast to
            # partition halves via one rank-2 matmul, one reciprocal for
            # both heads, then two multiplies.
            isl = slice(ic * TC, (ic + 1) * TC)
            dz = dnz[(hp // 2 + ic) % 2]
            nc.vector.tensor_copy(dz[0:1, :], po_a[HD:HD + 1, :])
            nc.vector.tensor_copy(dz[64:65, :], po_b[HD:HD + 1, :])
            prb = sp.tile([128, TC], F32, name="prb", tag="pf", bufs=2)
            nc.tensor.matmul(prb[:], e0z2[:], dz[:], start=True, stop=True)
            rb = wp.tile([128, TC], F32, name="rb", tag="rb", bufs=2)
            nc.vector.reciprocal_approx_fast(rb[:], prb[:])
            nc.vector.tensor_mul(ao[hp // 2][0:HD, isl],
                                 po_a[0:HD, :], rb[0:HD, :])
            nc.vector.tensor_mul(ao[hp // 2][HD:128, isl],
                                 po_b[0:HD, :], rb[HD:128, :])

        # The t4=ic+1 projection chunk is emitted right after attention(ic)
        # so the scheduler can pull its matmuls forward into attention's PE
        # stall gaps (keeps the PE dense and the HAM clock-gate open).
        proj_chunk(0, xt=xt0)
        vp_build(0)
        # deferred low-urgency loads (first needed at norm / out-proj of
        # ic=0) so the startup DMA burst prioritizes x and the projection
        # weights
        for i in range(2):
            nc.sync.dma_start(out=owt[i][:], in_=dr["owT"][i * 128:(i + 1) * 128, :])
        nc.sync.dma_start(out=e0z2[:], in_=dr["e0z2"])
        for i in range(2):
            nc.sync.dma_start(out=dnz[i][:], in_=dr["zz"])
        for ic in range(NTC):
            njb = 4 * (ic + 1)
            sl = slice(ic * TC, (ic + 1) * TC)
            if ic + 1 < NTC:
                proj_chunk(ic + 1)
                vp_build(ic + 1)
            for hp in (0, 2):
                po_a = sp.tile([HD + 1, TC], F32, name="po_a", tag="po", bufs=2)
                po_b = sp.tile([HD + 1, TC], F32, name="po_b", tag="po", bufs=2)
                for g in range(2 * ic):
                    t1p_a = wp.tile([128, 2 * TC], SC, name="t1p_a",
                                    tag="t1p", bufs=6)
                    eep_a = wp.tile([128, 2 * TC], SC, name="eep_a",
                                    tag="eep", bufs=6)
                    t1p_b = wp.tile([128, 2 * TC], SC, name="t1p_b",
                                    tag="t1p", bufs=6)
                    eep_b = wp.tile([128, 2 * TC], SC, name="eep_b",
                                    tag="eep", bufs=6)
                    attn_tile(hp, ic, 2 * g, po_a, (t1p_a, eep_a, 0))
                    attn_tile(hp + 1, ic, 2 * g, po_b, (t1p_b, eep_b, 0))
                    attn_tile(hp, ic, 2 * g + 1, po_a, (t1p_a, eep_a, 1))
                    attn_tile(hp + 1, ic, 2 * g + 1, po_b, (t1p_b, eep_b, 1))
                    attn_av(hp, ic, 2 * g, po_a, eep_a[:, 0:TC])
                    attn_av(hp + 1, ic, 2 * g, po_b, eep_b[:, 0:TC])
                    attn_av(hp, ic, 2 * g + 1, po_a, eep_a[:, TC:2 * TC])
                    attn_av(hp + 1, ic, 2 * g + 1, po_b, eep_b[:, TC:2 * TC])
                for q in range(4):
                    attn_tile(hp, ic, 4 * ic + q, po_a, None)
                    attn_tile(hp + 1, ic, 4 * ic + q, po_b, None)
                attn_norm(hp, ic, po_a, po_b)
            # output projection for this t-chunk (all heads now done)
            for ec in range(D // 128):
                esl = slice(ec * 128, (ec + 1) * 128)
                pf = sp.tile([128, TC], F32, name="pf", tag="pf", bufs=2)
                nc.tensor.matmul(pf[:], owt[0][:, esl], ao[0][:, sl],
                                 start=True, stop=False)
                nc.tensor.matmul(pf[:], owt[1][:, esl], ao[1][:, sl],
                                 start=False, stop=True)
                fo = wp.tile([128, TC], SC, name="fo", tag="fo", bufs=8)
                if ec % 2 == 0:
                    nc.vector.tensor_copy(fo[:], pf[:])
                else:
                    nc.scalar.copy(fo[:], pf[:])
                nc.sync.dma_start(out=dr["poutT"][esl, sl], in_=fo[:])


def _build(alpha, inv_beta):
    SC = BF16 if USE_BF16 else F32R
    nc = bacc.Bacc("TRN2", debug=False)
    dr = {}
    dr["xT"] = nc.dram_tensor("xT", [D, T], SC, kind="ExternalInput").ap()
    dr["wqkvT"] = nc.dram_tensor(
        "wqkvT", [D, 3 * HPC * HD], SC, kind="ExternalInput").ap()
    dr["whqT"] = nc.dram_tensor("whqT", [D, 128], SC, kind="ExternalInput").ap()
    dr["bT"] = nc.dram_tensor("bT", [128, 128], SC, kind="ExternalInput").ap()
    dr["owT"] = nc.dram_tensor(
        "owT", [HPC * HD, D], F32R, kind="ExternalInput").ap()
    dr["ident"] = nc.dram_tensor("ident", [128, 128], SC, kind="ExternalInput").ap()
    dr["trinegT"] = nc.dram_tensor("trinegT", [128, 128], SC, kind="ExternalInput").ap()
    dr["e0z2"] = nc.dram_tensor("e0z2", [128, 128], F32R, kind="ExternalInput").ap()
    dr["zz"] = nc.dram_tensor("zz", [128, TC], F32R, kind="ExternalInput").ap()
    dr["poutT"] = nc.dram_tensor("poutT", [D, T], SC, kind="ExternalOutput").ap()
    with tile.TileContext(nc) as tc_:
        _emit(nc, tc_, dr, alpha, inv_beta)
    nc.compile()
    return nc


def _sigmoid(v):
    return 1.0 / (1.0 + np.exp(-v))


def _round_f32r(a):
    """Round fp32 -> fp32r bit pattern (11-bit mantissa, rte)."""
    u = np.ascontiguousarray(a, np.float32).view(np.uint32)
    r = (u + 0x7FF + ((u >> 12) & 1)) & np.uint32(0xFFFFF000)
    return r.view(np.float32)


def _sc_cast(a):
    """Cast an fp32 array to the score-chain wire dtype."""
    a = np.ascontiguousarray(a, np.float32)
    if USE_BF16:
        return a.astype(ml_dtypes.bfloat16)
    return _round_f32r(a)


def _host_prep(x, qkv_w, out_w, hex_w, hamming_lambda_logit, q6_w,
               transforms, transform_weights, scale_logit, sips_alpha,
               sips_beta):
    """Build the per-core input maps (all host work is slicing/transposes)."""
    x = np.asarray(x, np.float32)
    qkv_w = np.asarray(qkv_w, np.float32)
    out_w = np.asarray(out_w, np.float32)
    hex_w = np.asarray(hex_w, np.float32)
    q6_w = np.asarray(q6_w, np.float32)
    transforms = np.asarray(transforms, np.float32)
    transform_weights = np.asarray(transform_weights, np.float32)

    lam = float(_sigmoid(np.float32(hamming_lambda_logit)))
    scale2 = float(_sigmoid(np.float32(scale_logit))) * 2.0
    alpha = float(np.asarray(sips_alpha).reshape(-1)[0])
    inv_beta = 1.0 / float(np.asarray(sips_beta).reshape(-1)[0])

    tw = np.asarray(transform_weights, np.float64) / TEMP
    w = np.exp(tw - tw.max(-1, keepdims=True))
    w = (w / w.sum(-1, keepdims=True)).astype(np.float32)      # (H, NT)
    Mh = np.einsum("ht,tde->hde", w, transforms)               # (H, 6, 6)

    whq1 = np.vstack([hex_w, q6_w]).T                          # (D, 12)
    whqT_f = np.zeros((D, 128), np.float32)
    for hh in range(4):
        whqT_f[:, 32 * hh:32 * hh + 12] = whq1
    whqT = _sc_cast(whqT_f)                                    # (D, 128)
    ident = _sc_cast(np.eye(128, dtype=np.float32))
    # trinegT[k, j] = -30 where query-offset k < key j (additive causal
    # mask for the diagonal blocks, applied on the PE)
    trinegT = _sc_cast(np.where(
        np.arange(128)[:, None] < np.arange(128)[None, :], -30.0, 0.0
    ).astype(np.float32))
    e0z2_h = np.zeros((128, 128), np.float32)
    e0z2_h[0, 0:HD] = 1.0
    e0z2_h[64, HD:128] = 1.0
    zz_h = np.zeros((128, TC), np.float32)
    bigB = np.zeros((H, 12, 12), np.float32)
    for h in range(H):
        bigB[h, :6, :6] = (lam / 2.0) * np.eye(6, dtype=np.float32)
        bigB[h, 6:, 6:] = (scale2 / 6.0) * Mh[h]

    in_maps = []
    for core in range(NCORES):
        b = core // CPB
        heads = [(core % CPB) * HPC + k for k in range(HPC)]
        rows = []
        for part in range(3):
            for h in heads:
                rows.extend(range(part * D + h * HD, part * D + (h + 1) * HD))
        wqkvT = _sc_cast(qkv_w[rows, :].T)                      # (D, 768)
        cols = []
        for h in heads:
            cols.extend(range(h * HD, (h + 1) * HD))
        owT = _round_f32r(out_w[:, cols].T)                     # (256, D)
        bT = np.zeros((128, 128), np.float32)                    # block-diag
        for hh in range(HPC):
            bT[32 * hh:32 * hh + 12, 32 * hh:32 * hh + 12] = bigB[heads[hh]].T
        in_maps.append({
            "xT": _sc_cast(x[b].T),
            "wqkvT": wqkvT,
            "whqT": whqT,
            "bT": _sc_cast(bT),
            "owT": owT,
            "ident": ident,
            "e0z2": e0z2_h,
            "zz": zz_h,
            "trinegT": trinegT,
        })
    return in_maps, alpha, inv_beta


_CACHE = {}
LAST_RESULT = None


def kernel(**inputs):
    global LAST_RESULT
    in_maps, alpha, inv_beta = _host_prep(**inputs)
    key = (round(alpha, 9), round(inv_beta, 9), USE_BF16)
    if key not in _CACHE:
        _CACHE[key] = _build(alpha, inv_beta)
    nc = _CACHE[key]
    res = run_bass_kernel_spmd(nc, in_maps, list(range(NCORES)))
    LAST_RESULT = res
    out = np.zeros((B, T, D), np.float32)
    for b in range(B):
        acc = np.zeros((D, T), np.float32)
        for core in range(b * CPB, (b + 1) * CPB):
            acc += np.asarray(res.results[core]["poutT"], dtype=np.float32)
        out[b] = acc.T
    return out

